# revision 46
# baseline (speedup 1.0000x reference)
"""Trainium2 Bass kernel for PVT-style spatial-reduction attention.

Problem (per batch element b of 8, one NeuronCore each — pure data parallel):
  q  = x @ Wq + bq                                  [16384, 64]
  xs = conv8x8s8(x.reshape(128,128,64), Wsr) + bsr  [256, 64]
  xs = LayerNorm(xs) * gamma + beta
  k  = xs @ Wk + bk ; v = xs @ Wv + bv              [256, 64]
  A  = softmax(q @ k.T / 8) ; o = A @ v             [16384, 64]
  out = o @ Wp + bp

v5 design (on top of the v4 host-folds/layout):
  - All weight-only folds on the HOST (gt2/wb/nxw/bvp/bsr/wsr2/g128); x
    host-pre-permuted into the on-chip xT2 layout in bf16:
      xT2[jp*64+c, (blk*8+pj)*128 + p] = x[blk*2048 + p*16 + pj*2 + jp, c]
  - wsr2 ships only the 32 even tap-pair slots actually loaded as
    stationary (half the v4 bytes); x is split into xlo/xhi tiles so
    conv half 0 starts as soon as the first 1MB lands.  All input DMAs
    issue from sync (HWDGE) and stay concurrent — serializing transfers
    loses aggregate bandwidth (~170GB/s single vs ~340 shared).
  - A bf16 junk-matmul bridge (~26 N=256 MMs into a PSUM bank that conv
    overwrites) keeps the HAM clock gate busy across the DMA wait so
    conv runs at 2.4GHz; one N=512 junk matmul every 4 chunks keeps it
    there through attention (without it the PE re-throttles, +2.8us).
  - LayerNorm fully per-half: every per-half intermediate has its OWN
    tile (Tile dependency tracking is tile-granular; sharing one tile
    across halves serializes half 0 behind half 1's writers).  Half 0's
    whole stats chain hides under conv half 1.  kq2 skips the xsn
    round-trip: kqraw = G@xs runs pre-LN during the stats chain, then
    kq2 = a*kqraw + gsum*b (two DVE ops; a|b broadcast by one K=1
    matmul, staged to SBUF so the PSUM bank frees for attention).
  - exp(S) split: ACT exact exp for ACT_COLS, DVE int16-Schraudolph for
    the rest: bf16_bits(e^(S/8)) ~= int16(S*23.083 + 16250.5); both
    engines measure ~0.9-1.0ns/col on PSUM-f32 input and pace the
    attention loop (~0.93-1.1us/chunk); the PE (~80% busy) does not.
  - S matmuls emitted mh-outer: consecutive par0/par1 K=64 matmuls sit
    on disjoint PE row groups + different PSUM banks and run
    concurrently.  y lags S/exp by TWO chunks so the in-order PE queue
    never waits on the current exp; y/norm emit BEFORE S(ci) so the
    norm mul runs under the S matmuls instead of queue-blocking the
    Schraudolph that gates the next s-bank release.
  - Softmax normalization batched over 2 chunks (the mul doubles as the
    mandatory PSUM->SBUF drain; GPSIMD and DMA have no PSUM access, so
    ACT/DVE own all elementwise work); the final group is normalized
    and DMA'd per-chunk to shorten the tail.  Output is bf16 (host
    casts back to f32).
"""

import os
import sys

import numpy as np

for _p in ("/root/.axon_site", "/root/.axon_site/_ro/trn_rl_repo",
           "/root/.axon_site/_ro/pypackages", "/opt/trn_rl_repo"):
    if os.path.isdir(_p) and _p not in sys.path:
        sys.path.append(_p)

import ml_dtypes  # noqa: E402

import concourse.bass as bass  # noqa: E402
import concourse.mybir as mybir  # noqa: E402
import concourse.tile as tile  # noqa: E402
from concourse import bacc  # noqa: E402
from concourse.bass_utils import run_bass_kernel_spmd  # noqa: E402
from concourse.masks import make_identity  # noqa: E402

F32 = mybir.dt.float32
F32R = mybir.dt.float32r
BF16 = mybir.dt.bfloat16
I16 = mybir.dt.int16
F8 = mybir.dt.float8e4
AF = mybir.ActivationFunctionType
ALU = mybir.AluOpType

N_CORES = 8
N = 16384          # tokens per core (H*W = 128*128)
C = 64             # channels
SR = 8
NKV = 256          # (128/8)^2
EPS = 1e-5
N_CHUNK = 512      # query tokens per attention chunk
N_CHUNKS = N // N_CHUNK  # 32
TOK_TILE = 128
NTAP = SR * SR // 2   # 32 packed tap-pairs (even tap on p<64, odd on p>=64)

# exp column split: ACT exact exp | DVE int16-Schraudolph.  Both engines
# measure ~0.9ns/col marginal; DVE also carries the softmax-normalize mul
# + reciprocal (~430ns/chunk amortized), so ACT takes the larger share.
# (GPSIMD cannot touch PSUM — neither directly nor via DMA staging — so it
# cannot help with any attention-phase work.)
ACT_COLS = 720
# Schraudolph: bf16_bits(exp(S/8)) ~= int16(S * (2^7/ln2)/8 + 127*2^7 - C)
EXP_A = 184.66496423378 / 8.0
EXP_B = 16250.5

NBF = 2 * C + 1 + C   # gt2 | wb | nxw columns in the bf16 const blob


def _patch_act_tables():
    """Bias the ACT-table-load insertion pass so Ln and Exp both resolve to
    the one act_info table that contains them both
    (natural_log_exp_and_others).  Table order/indices are untouched (walrus
    maps act_func_set_id by index); we only narrow the pass's coverage view,
    so the single load happens at the early dummy Ln and nothing reloads on
    the critical path."""
    from concourse import hw_specs
    orig = hw_specs.get_activation_tables

    def patched(arch):
        t = {name: set(fns) for name, fns in orig(arch).items()}
        for name, fns in t.items():
            if name != "natural_log_exp_and_others":
                fns.discard(AF.Exp)
                fns.discard(AF.Ln)
        return t

    bacc.get_activation_tables = patched


def build_graph():
    _patch_act_tables()
    nc = bacc.Bacc("TRN2", target_bir_lowering=False, debug=False,
                   num_devices=N_CORES)

    xt2_ext = nc.declare_dram_parameter("xt2", [128, N // 2], BF16,
                                        isOutput=False)
    wbf_ext = nc.declare_dram_parameter("wbf", [C, NBF], BF16, isOutput=False)
    wf32_ext = nc.declare_dram_parameter("wf32", [C, 2], F32, isOutput=False)
    g128_ext = nc.declare_dram_parameter("g128", [128, 1], F32, isOutput=False)
    wsr2_ext = nc.declare_dram_parameter("wsr2", [128, NTAP * C], BF16,
                                         isOutput=False)
    out_ext = nc.declare_dram_parameter("out", [N, C], BF16, isOutput=True)

    with tile.TileContext(nc) as tc:
        with tc.tile_pool(name="const", bufs=1) as const_pool, \
             tc.tile_pool(name="persist", bufs=1) as persist_pool, \
             tc.tile_pool(name="work", bufs=2) as work_pool:

            # ---------- constants first: gpsimd/scalar queues start hot ----
            identity = const_pool.tile([128, 128], F32)
            make_identity(nc, identity[:])

            eps_t = const_pool.tile([1, 1], F32, tag="eps")
            nc.gpsimd.memset(eps_t[:], EPS)
            # dummy Ln: pre-loads the natural_log table early; the LN-phase
            # rstd=exp(-0.5*ln(var)) then pays exactly one table swap (to
            # exp), after which attention needs no more loads.
            warm_t = const_pool.tile([1, 1], F32, tag="warm")
            nc.scalar.activation(warm_t[:], eps_t[:], AF.Ln)

            junk_mv = const_pool.tile([128, 256], BF16, tag="junkmv")
            nc.gpsimd.memset(junk_mv[:], 1.0)
            junk_st = const_pool.tile([128, 128], BF16, tag="junkst")
            nc.gpsimd.memset(junk_st[:], 0.001)

            # stats stationary: 65 rows of 1/C (row 64 weights the eps row)
            ones65_st = const_pool.tile([C + 1, 1], F32, tag="ones65_st")
            nc.gpsimd.memset(ones65_st[:], 1.0 / C)
            ones65 = const_pool.tile([C + 1, 1], BF16, tag="ones65")
            nc.vector.tensor_copy(ones65[:], ones65_st[:])
            onesr_st = const_pool.tile([1, 128], F32, tag="onesr_st")
            nc.gpsimd.memset(onesr_st[:], 1.0)
            onesr1 = const_pool.tile([1, 128], BF16, tag="onesr1")
            nc.vector.tensor_copy(onesr1[:], onesr_st[:])

            # ---------- DMAs (all HWDGE via sync; 1 descriptor/partition) --
            # All transfers go out together: a single in-flight transfer
            # only sustains ~170GB/s, so serializing (gating) LOSES
            # aggregate bandwidth.  Concurrent, everything lands ~15us.
            HD = N // 4  # 4096 cols per x half (conv blocks 0-3 / 4-7)
            xlo = persist_pool.tile([128, HD], BF16, tag="xlo")
            xhi = persist_pool.tile([128, HD], BF16, tag="xhi")
            wsr_sb = const_pool.tile([128, NTAP, C], BF16, tag="wsr")
            wbf_sb = const_pool.tile([C, NBF], BF16, tag="wbf")
            wf32_sb = const_pool.tile([C, 2], F32, tag="wf32")
            g128_sb = const_pool.tile([128, 1], F32, tag="g128")
            nc.sync.dma_start(wsr_sb[:].rearrange("p t c -> p (t c)"),
                              wsr2_ext[:])
            nc.sync.dma_start(xlo[:], xt2_ext[:, 0:HD])
            nc.sync.dma_start(wbf_sb[:], wbf_ext[:])
            nc.sync.dma_start(wf32_sb[:], wf32_ext[:])
            nc.sync.dma_start(xhi[:], xt2_ext[:, HD:2 * HD])
            nc.sync.dma_start(g128_sb[:], g128_ext[:])

            gt2_sb = wbf_sb[:, 0:2 * C]
            wb_sb = wbf_sb[:, 2 * C:2 * C + 1]
            nxw_sb = wbf_sb[:, 2 * C + 1:NBF]
            bvp_sb = wf32_sb[:, 0:1]
            bsr_sb = wf32_sb[:, 1:2]

            xhv = [x[:].rearrange(
                "p (b jp dh i1 di jh) -> p b jp dh i1 di jh",
                b=4, jp=2, dh=4, i1=2, di=8, jh=8) for x in (xlo, xhi)]

            ov = out_ext[:].rearrange("(b p ur j) f -> b p ur j f",
                                      b=8, p=TOK_TILE, ur=8, j=2)

            vps = [persist_pool.tile([TOK_TILE, C + 1], BF16, tag=f"vps{h}",
                                     name=f"vps{h}")
                   for h in range(2)]

            with tc.tile_pool(name="pre_psum", bufs=2, space="PSUM") as pre_ps:
                # Every per-half intermediate gets its OWN tile: the Tile
                # dependency tracker is tile-granular, so sharing one tile
                # across halves falsely serializes half 0's chain behind
                # half 1's writers (measured: +2.5us on the LN tail).
                xs_ps = [pre_ps.tile([C, 128], F32, tag=f"conv{h}", bufs=1,
                                     name=f"conv{h}") for h in range(2)]
                kqr_ps = [pre_ps.tile([128, 128], F32, tag=f"kqr{h}", bufs=1,
                                      name=f"kqr{h}") for h in range(2)]
                m12_ps = [pre_ps.tile([1, 2, 128], F32, tag=f"m12{h}", bufs=1,
                                      name=f"m12{h}") for h in range(2)]
                ab_ps = [pre_ps.tile([128, 2, 128], F32, tag=f"abp{h}",
                                     bufs=1, name=f"abp{h}") for h in range(2)]

                # PE warm-up: a bf16 junk-matmul bridge (into abp[0], which
                # is overwritten later anyway) keeps the HAM clock gate busy
                # from engine start until conv's DMA data lands, so conv
                # runs at 2.4 GHz.  (f32 junk lowers to 2 HW passes and ran
                # cold-paced; bf16 N=256 is one 213ns-cold/107ns-warm pass.)
                junk_out = ab_ps[0][:].rearrange("p s b -> p (s b)")
                for _ in range(26):
                    nc.tensor.matmul(junk_out, junk_st[:], junk_mv[:],
                                     start=True, stop=True)

                def conv_half(bh):
                    # taps over x half bh -> kv columns [128bh, +128)
                    xv = xhv[bh]
                    for k, dj in enumerate(range(0, SR, 2)):
                        for di in range(SR):
                            tap = di * (SR // 2) + dj // 2
                            nc.tensor.matmul(
                                xs_ps[bh][:],
                                wsr_sb[:, tap, :],
                                xv[:, :, :, dj // 2, :, di, :],
                                start=(k == 0 and di == 0),
                                stop=(k == SR // 2 - 1 and di == SR - 1))

                # ---------- layernorm, fully per-half pipelined ----------
                # xs2_h = [xs | xs^2] bf16; row 64 = [0 | eps*C] so the 1/C
                # stationary yields m12_h = [mu | E[x^2]+eps] in one matmul.
                # Half 0's whole chain runs while conv half 1 is on the PE.
                xs2 = [work_pool.tile([C + 1, 2, 128], BF16, tag=f"sq{h}",
                                      name=f"sq{h}") for h in range(2)]
                kqraw = [work_pool.tile([128, 128], BF16, tag=f"kqraw{h}",
                                        name=f"kqraw{h}") for h in range(2)]
                ab = [work_pool.tile([1, 2, 128], BF16, tag=f"ab{h}",
                                     name=f"ab{h}") for h in range(2)]
                kq2h = [persist_pool.tile([128, 128], BF16, tag=f"kq2{h}",
                                          name=f"kq2{h}") for h in range(2)]
                xsn = work_pool.tile([C, NKV], BF16, tag="xsn")
                for h in range(2):
                    nc.gpsimd.memset(xs2[h][C:C + 1, 0, :], 0.0)
                    nc.gpsimd.memset(xs2[h][C:C + 1, 1, :], EPS * C)

                def ln_front(h):
                    # emitted right after conv_half(h): xs/xs^2 + the two
                    # PE ops (kqraw projection, stats matmul)
                    nc.vector.tensor_scalar_add(xs2[h][0:C, 0, :],
                                                xs_ps[h][:], bsr_sb)
                    nc.scalar.activation(xs2[h][0:C, 1, :], xs_ps[h][:],
                                         AF.Square, bias=bsr_sb)
                    # kqraw = G @ xs (pre-LN); the LN affine lands later via
                    # kq2 = a*kqraw + gsum*b  (gsum = row-sums of G)
                    nc.tensor.matmul(kqr_ps[h][:], gt2_sb, xs2[h][0:C, 0, :],
                                     start=True, stop=True)
                    # h0's copy rides the idle ACT during conv half 1; h1's
                    # goes to DVE so it doesn't delay ACT's h1 stats chain.
                    if h == 0:
                        nc.scalar.copy(kqraw[h][:], kqr_ps[h][:])
                    else:
                        nc.vector.tensor_copy(kqraw[h][:], kqr_ps[h][:])
                    nc.tensor.matmul(
                        m12_ps[h][:].rearrange("p s b -> p (s b)"), ones65[:],
                        xs2[h][:].rearrange("p s b -> p (s b)"),
                        start=True, stop=True)

                def ln_mid(h):
                    # mu^2 -> var -> rstd=exp(-.5 ln var) -> b=-mu*rstd
                    mu2 = work_pool.tile([1, 128], F32, tag=f"mu2{h}",
                                         name=f"mu2{h}")
                    nc.scalar.activation(mu2[:], m12_ps[h][:, 0, :],
                                         AF.Square)
                    var = work_pool.tile([1, 128], F32, tag=f"var{h}",
                                         name=f"var{h}")
                    nc.vector.scalar_tensor_tensor(
                        var[:], m12_ps[h][:, 1, :], 1.0, mu2[:],
                        op0=ALU.mult, op1=ALU.subtract)
                    lnv = work_pool.tile([1, 128], F32, tag=f"lnv{h}",
                                         name=f"lnv{h}")
                    nc.scalar.activation(lnv[:], var[:], AF.Ln)
                    nc.scalar.activation(ab[h][:, 0, :], lnv[:], AF.Exp,
                                         scale=-0.5)
                    nc.vector.scalar_tensor_tensor(
                        ab[h][:, 1, :], m12_ps[h][:, 0, :], -1.0,
                        ab[h][:, 0, :], op0=ALU.mult, op1=ALU.mult)

                def ln_tail(h):
                    # broadcast a|b to 128 partitions; stage to SBUF at once
                    # (ACT, idle here) so the PSUM bank frees immediately --
                    # the attention pools reuse these banks, and a late
                    # reader of abp blocks the first S matmul on a bank WAR.
                    if h == 1:
                        # dep-timed junk: executes when ab[1] lands (~right
                        # before ab2-1), bridging the LN-tail PE idle so the
                        # first attention chunks run at 2.4 GHz
                        nc.tensor.matmul(kqr_ps[1][:], onesr1[0:1, :],
                                         ab[1][0:1, 0, :],
                                         start=True, stop=True)
                    nc.tensor.matmul(
                        ab_ps[h][:].rearrange("p s b -> p (s b)"), onesr1[:],
                        ab[h][:].rearrange("p s b -> p (s b)"),
                        start=True, stop=True)
                    t1 = work_pool.tile([128, 128], F32, tag=f"t1kq{h}",
                                        name=f"t1kq{h}")
                    nc.vector.tensor_mul(t1[:], kqraw[h][:], ab_ps[h][:, 0, :])
                    ab_sb = work_pool.tile([128, 2, 128], BF16,
                                           tag=f"absb{h}", name=f"absb{h}")
                    nc.scalar.copy(ab_sb[:], ab_ps[h][:])
                    nc.vector.scalar_tensor_tensor(
                        kq2h[h][:], ab_ps[h][:, 1, :], g128_sb[:], t1[:],
                        op0=ALU.mult, op1=ALU.add)
                    cs = slice(128 * h, 128 * h + 128)
                    nc.vector.tensor_mul(xsn[:, cs], xs2[h][0:C, 0, :],
                                         ab_sb[0:C, 0, :])
                    nc.vector.tensor_add(xsn[:, cs], xsn[:, cs],
                                         ab_sb[0:C, 1, :])
                    if h == 0:
                        # free-running junks keep the PE's HAM activity up
                        # through the serial h1 stats chain
                        for _ in range(6):
                            nc.tensor.matmul(kqr_ps[0][:], junk_st[:],
                                             junk_mv[:, 0:128],
                                             start=True, stop=True)

                conv_half(0)
                ln_front(0)
                conv_half(1)
                ln_mid(0)
                ln_front(1)
                ln_tail(0)
                ln_mid(1)
                ln_tail(1)

            # ---------- attention ----------
            # chunk ci = xT2 free block [256ci, 256ci+256): even-parity tokens
            # on partitions 0:64, odd on 64:128.  E col layout per chunk:
            # par*512 + mh*256 + tok.  y(ci-1) is emitted after S(ci)/exp(ci)
            # so the in-order PE queue never waits on exp before S(ci+1).
            def emit_s_exp(ci, s_pool, e_pool):
                s_ps = s_pool.tile([TOK_TILE, 2 * N_CHUNK], F32, tag="S")
                xh = (xlo, xhi)[ci // 16]
                xb = xh[:, 256 * (ci % 16):256 * (ci % 16 + 1)]
                # mh-outer: consecutive par0/par1 matmuls sit on disjoint PE
                # row groups (tile_position rows 0/64 via base_partition) and
                # write different PSUM banks, so each pair runs concurrently.
                for mh in range(2):
                    for par in range(2):   # bank `par`: tokens of parity par
                        o = C * par
                        base = par * N_CHUNK + mh * 256
                        nc.tensor.matmul(s_ps[:, base:base + 256],
                                         kq2h[mh][o:o + C, :],
                                         xb[o:o + C, :], start=True, stop=True)
                e_t = e_pool.tile([TOK_TILE, 2 * N_CHUNK], BF16, tag="E",
                                  bufs=5)
                nc.scalar.activation(e_t[:, 0:ACT_COLS], s_ps[:, 0:ACT_COLS],
                                     AF.Exp, scale=0.125)
                nc.vector.tensor_scalar(
                    e_t[:, ACT_COLS:2 * N_CHUNK].bitcast(I16),
                    s_ps[:, ACT_COLS:2 * N_CHUNK],
                    EXP_A, EXP_B, op0=ALU.mult, op1=ALU.add)
                return e_t

            def emit_vps_chain(scratch):
                """V-path: PSUM carved from `scratch` (one free y bank)."""
                d_h = []
                for h in range(2):
                    bqk_ps = scratch[:, 384 + h:385 + h]
                    nc.tensor.matmul(bqk_ps,
                                     xsn[:, h * 128:(h + 1) * 128],
                                     wb_sb, start=True, stop=True)
                    dh = work_pool.tile([TOK_TILE, 1], F32, tag="dh")
                    nc.scalar.activation(dh[:], bqk_ps, AF.Exp, scale=0.125)
                    d_h.append(dh)
                # all vector work on ACT (Identity with bias/scale APs): the
                # DVE queue stays clear for the chunk-0..2 Schraudolphs, so
                # the s_ps ring releases on time during the pipeline ramp.
                vpT_ps = scratch[0:C, 128:384]
                nc.tensor.matmul(vpT_ps, nxw_sb, xsn[:], start=True, stop=True)
                vpT = work_pool.tile([C, NKV], F32, tag="vT_b")
                nc.vector.tensor_scalar_add(vpT[:], vpT_ps, bvp_sb)
                for h in range(2):
                    vpt_ps = scratch[:, 64 * h:64 * (h + 1)]
                    nc.tensor.transpose(vpt_ps, vpT[:, h * 128:(h + 1) * 128],
                                        identity[0:C, 0:C])
                    nc.vector.tensor_scalar(vps[h][:, 0:C], vpt_ps,
                                            d_h[h][:], None, op0=ALU.mult)
                    nc.vector.tensor_copy(vps[h][:, C:C + 1], d_h[h][:])

            def emit_y(ci, y_ps):
                e_t = e_tiles[ci]
                for u in range(4):
                    ysl = y_ps[:, ci % 2, u * (C + 1):(u + 1) * (C + 1)]
                    b, j = u // 2, u % 2
                    col0 = 512 * j + 128 * b
                    nc.tensor.matmul(ysl, e_t[:, col0:col0 + 128],
                                     vps[0][:], start=True, stop=False)
                    nc.tensor.matmul(ysl, e_t[:, 256 + col0:256 + col0 + 128],
                                     vps[1][:], start=False, stop=True)

            def emit_norm(ci, y_ps):
                # normalize chunks (ci-1, ci) and DMA out
                yv = y_ps[:, :, 0:4 * (C + 1)].rearrange(
                    "p t (u q) -> p t u q", u=4)
                r_t = work_pool.tile([TOK_TILE, 2, 4, 1], F32, tag="r", bufs=2)
                nc.vector.reciprocal(r_t[:], yv[:, :, :, C:C + 1])
                y_t = work_pool.tile([TOK_TILE, 8, C], BF16, tag="y", bufs=2)
                nc.vector.tensor_mul(
                    y_t[:].rearrange("p (t u) f -> p t u f", t=2),
                    yv[:, :, :, 0:C],
                    r_t[:].broadcast_to([TOK_TILE, 2, 4, C]))
                g = ci // 2
                nc.sync.dma_start(
                    ov[g // 2, :, 4 * (g % 2):4 * (g % 2) + 4, :, :],
                    y_t[:].rearrange("p (s j) f -> p s j f", s=4, j=2))

            # y lags S/exp by TWO chunks: y(ci-2)'s E finished a whole chunk
            # ago, so the in-order PE queue never stalls on the current exp.
            # PSUM: 2 S tiles (4 banks) + ~2 y tiles (4 banks) = 8.
            with tc.tile_pool(name="attn_psum_s", bufs=2, space="PSUM") as att_s, \
                 tc.tile_pool(name="attn_psum_y", bufs=1, space="PSUM") as att_y:
                e_tiles = {}
                y_tiles = {}

                def y_tile(k):
                    if k not in y_tiles:
                        y_tiles[k] = att_y.tile([TOK_TILE, 2, 512], F32,
                                                tag="Y", name=f"y{k}")
                    return y_tiles[k]

                def emit_y_norm(k):
                    emit_y(k, y_tile(k // 2))
                    if k % 2 == 1:
                        emit_norm(k, y_tile(k // 2))

                def emit_norm_single(k, y_ps):
                    # tail-only: normalize+DMA one chunk so the final DMA
                    # isn't serialized behind a 2-chunk norm
                    t = k % 2
                    yv = y_ps[:, t:t + 1, 0:4 * (C + 1)].rearrange(
                        "p t (u q) -> p t u q", u=4)
                    r_t = work_pool.tile([TOK_TILE, 1, 4, 1], F32, tag="r1",
                                         bufs=2)
                    nc.vector.reciprocal(r_t[:], yv[:, :, :, C:C + 1])
                    y_t = work_pool.tile([TOK_TILE, 4, C], BF16, tag="y1",
                                         bufs=2)
                    nc.vector.tensor_mul(
                        y_t[:].rearrange("p (t u) f -> p t u f", t=1),
                        yv[:, :, :, 0:C],
                        r_t[:].broadcast_to([TOK_TILE, 1, 4, C]))
                    g = k // 2
                    nc.sync.dma_start(
                        ov[g // 2, :,
                           4 * (g % 2) + 2 * t:4 * (g % 2) + 2 * t + 2, :, :],
                        y_t[:].rearrange("p (s j) f -> p s j f", s=2, j=2))

                for ci in range(N_CHUNKS):
                    # y/norm of ci-2 go FIRST: their E is long done, the PE
                    # queue can't stall, and the norm mul runs on DVE while
                    # the PE computes S(ci) -- emitted after S it queue-
                    # blocks schrau(ci+1) and the next s-bank release.
                    if ci >= 2:
                        if ci % 4 == 0:
                            # periodic junk burst for the HAM clock gate
                            # (without it the PE re-throttles, +2.8us).  It
                            # writes the y bank -- freed 2 chunks ago and
                            # overwritten by the y matmuls' start=True -- so
                            # unlike an s_ps target it never waits the late
                            # s-bank release nor delays S(ci).
                            k = ci - 2
                            nc.tensor.matmul(
                                y_tile(k // 2)[:, k % 2, 0:256], kq2h[0][:],
                                xlo[:, 0:256], start=True, stop=True)
                        emit_y_norm(ci - 2)
                    e_tiles[ci] = emit_s_exp(ci, att_s, work_pool)
                    if ci == 0:
                        # vps chain uses the (empty) second bank of y tile 0
                        emit_vps_chain(y_tile(0)[:, 1, :])
                    e_tiles.pop(ci - 3, None)
                k = N_CHUNKS - 2
                emit_y(k, y_tile(k // 2))
                emit_norm_single(k, y_tile(k // 2))
                emit_y(k + 1, y_tile(k // 2))
                emit_norm_single(k + 1, y_tile(k // 2))

    nc.finalize()
    return nc


_NC_CACHE = None


def _get_nc():
    global _NC_CACHE
    if _NC_CACHE is None:
        _NC_CACHE = build_graph()
    return _NC_CACHE


def _fold_weights(inputs):
    """Host-side weight folding (all pure functions of the weights)."""
    f32 = np.float32
    Wq = np.asarray(inputs["Wq"], f32)
    Wk = np.asarray(inputs["Wk"], f32)
    Wv = np.asarray(inputs["Wv"], f32)
    Wp = np.asarray(inputs["Wp"], f32)
    Wsr = np.asarray(inputs["Wsr"], f32)
    bq = np.asarray(inputs["bq"], f32)
    bv = np.asarray(inputs["bv"], f32)
    bsr = np.asarray(inputs["bsr"], f32)
    bp = np.asarray(inputs["bp"], f32)
    gamma = np.asarray(inputs["gamma"], f32)
    beta = np.asarray(inputs["beta"], f32)

    bf = ml_dtypes.bfloat16
    Wkg = gamma[:, None] * Wk
    Wvg = gamma[:, None] * Wv
    G = Wq @ Wkg.T                                   # [C, C]
    wbf = np.concatenate(
        [G.T, G.T, (Wkg @ bq)[:, None], Wvg @ Wp], axis=1)
    wbf = np.ascontiguousarray(wbf, dtype=bf)        # [C, 2C+1+C]
    bvp = (beta @ Wv + bv) @ Wp + bp
    wf32 = np.ascontiguousarray(
        np.stack([bvp, bsr], axis=1), dtype=f32)     # [C, 2]
    # gsum[p] = sum_c gt2[c, p] = row-sums of G (for the kq2 LN-affine fold)
    g128 = np.ascontiguousarray(
        np.tile(G.sum(axis=1), 2)[:, None], dtype=f32)  # [128, 1]

    # wsr2: packed tap-pairs — slot k holds even tap 2k on partitions 0:64
    # and its odd partner 2k+1 on partitions 64:128, so K=128 matmuls fuse
    # tap pairs across the token-parity partition split (only even taps are
    # ever loaded as stationary; the old layout shipped 2x the bytes).
    wsr_f = Wsr.reshape(SR * SR, C, C)               # [tap, cin, cout]
    wsr2 = np.zeros((128, NTAP, C), dtype=bf)
    wsr2[0:C] = np.swapaxes(wsr_f[0::2], 0, 1)       # [cin, pair, cout]
    wsr2[C:128] = np.swapaxes(wsr_f[1::2], 0, 1)
    wsr2 = np.ascontiguousarray(wsr2.reshape(128, NTAP * C))
    return dict(wbf=wbf, wf32=wf32, wsr2=wsr2, g128=g128)


def _make_in_maps(inputs):
    x = np.asarray(inputs["x"], dtype=np.float32)
    B = x.shape[0]
    assert x.shape == (B, N, C) and B == N_CORES, x.shape
    common = _fold_weights(inputs)
    x_bf = np.asarray(x, dtype=ml_dtypes.bfloat16)
    in_maps = []
    for i in range(N_CORES):
        # xT2[jp*64+c, (blk*8+pj)*128+p] = x[blk*2048 + p*16 + pj*2 + jp, c]
        x3 = x_bf[i].reshape(8, 128, 8, 2, C)        # [blk, p, pj, jp, c]
        xt2 = np.ascontiguousarray(
            x3.transpose(3, 4, 0, 2, 1).reshape(128, N // 2))
        in_maps.append(dict(common, xt2=xt2))
    return in_maps


def run(inputs, trace=False):
    nc = _get_nc()
    in_maps = _make_in_maps(inputs)
    res = run_bass_kernel_spmd(nc, in_maps, list(range(N_CORES)), trace=trace)
    out = np.stack([np.asarray(res.results[i]["out"]) for i in range(N_CORES)])
    return out.astype(np.float32), res


def kernel(**inputs):
    out, _ = run(inputs, trace=False)
    return out



# revision 48
# speedup vs baseline: 1.1414x; 1.1414x over previous
"""Trainium2 Bass kernel for PVT-style spatial-reduction attention.

Problem (per batch element b of 8, one NeuronCore each — pure data parallel):
  q  = x @ Wq + bq                                  [16384, 64]
  xs = conv8x8s8(x.reshape(128,128,64), Wsr) + bsr  [256, 64]
  xs = LayerNorm(xs) * gamma + beta
  k  = xs @ Wk + bk ; v = xs @ Wv + bv              [256, 64]
  A  = softmax(q @ k.T / 8) ; o = A @ v             [16384, 64]
  out = o @ Wp + bp

v5 design (on top of the v4 host-folds/layout):
  - All weight-only folds on the HOST (gt2/wb/nxw/bvp/bsr/wsr2/g128); x
    host-pre-permuted into the on-chip xT2 layout in bf16:
      xT2[jp*64+c, (blk*8+pj)*128 + p] = x[blk*2048 + p*16 + pj*2 + jp, c]
  - wsr2 ships only the 32 even tap-pair slots actually loaded as
    stationary (half the v4 bytes); x is split into xlo/xhi tiles so
    conv half 0 starts as soon as the first 1MB lands.  All input DMAs
    issue from sync (HWDGE) and stay concurrent — serializing transfers
    loses aggregate bandwidth (~170GB/s single vs ~340 shared).
  - A bf16 junk-matmul bridge (~26 N=256 MMs into a PSUM bank that conv
    overwrites) keeps the HAM clock gate busy across the DMA wait so
    conv runs at 2.4GHz; junk MMs threaded through the LN tail plus one
    N=256 junk matmul every 4 chunks keep it there through attention
    (without them the PE re-throttles, +2.8us).
  - LayerNorm fully per-half: every per-half intermediate has its OWN
    tile (Tile dependency tracking is tile-granular; sharing one tile
    across halves serializes half 0 behind half 1's writers).  Half 0's
    whole stats chain hides under conv half 1.  kq2 skips the xsn
    round-trip: kqraw = G@xs runs pre-LN during the stats chain, then
    kq2 = a*kqraw + gsum*b (two DVE ops; a|b broadcast by one K=1
    matmul, staged to SBUF so the PSUM bank frees for attention).
  - exp(S) split: ACT exact exp for ACT_COLS, DVE int16-Schraudolph for
    the rest: bf16_bits(e^(S/8)) ~= int16(S*23.083 + 16250.5); both
    engines measure ~0.9-1.0ns/col on PSUM-f32 input and pace the
    attention loop (~0.93-1.1us/chunk); the PE (~80% busy) does not.
  - S matmuls emitted mh-outer: consecutive par0/par1 K=64 matmuls sit
    on disjoint PE row groups + different PSUM banks and run
    concurrently.  y lags S/exp by TWO chunks so the in-order PE queue
    never waits on the current exp; y/norm emit BEFORE S(ci) so the
    norm mul runs under the S matmuls instead of queue-blocking the
    Schraudolph that gates the next s-bank release.
  - Softmax normalization batched over 2 chunks (the mul doubles as the
    mandatory PSUM->SBUF drain; GPSIMD and DMA have no PSUM access, so
    ACT/DVE own all elementwise work); the final group is normalized
    and DMA'd per-chunk to shorten the tail.  Output is bf16 (host
    casts back to f32).
"""

import os
import sys

import numpy as np

for _p in ("/root/.axon_site", "/root/.axon_site/_ro/trn_rl_repo",
           "/root/.axon_site/_ro/pypackages", "/opt/trn_rl_repo"):
    if os.path.isdir(_p) and _p not in sys.path:
        sys.path.append(_p)

import ml_dtypes  # noqa: E402

import concourse.bass as bass  # noqa: E402
import concourse.mybir as mybir  # noqa: E402
import concourse.tile as tile  # noqa: E402
from concourse import bacc  # noqa: E402
from concourse.bass_utils import run_bass_kernel_spmd  # noqa: E402
from concourse.masks import make_identity  # noqa: E402

F32 = mybir.dt.float32
F32R = mybir.dt.float32r
BF16 = mybir.dt.bfloat16
I16 = mybir.dt.int16
F8 = mybir.dt.float8e4
AF = mybir.ActivationFunctionType
ALU = mybir.AluOpType

N_CORES = 8
N = 16384          # tokens per core (H*W = 128*128)
C = 64             # channels
SR = 8
NKV = 256          # (128/8)^2
EPS = 1e-5
N_CHUNK = 512      # query tokens per attention chunk
N_CHUNKS = N // N_CHUNK  # 32
TOK_TILE = 128
NTAP = SR * SR // 2   # 32 packed tap-pairs (even tap on p<64, odd on p>=64)

# exp column split: ACT exact exp | DVE int16-Schraudolph.  Both engines
# measure ~0.9ns/col marginal; DVE also carries the softmax-normalize mul
# + reciprocal (~430ns/chunk amortized), so ACT takes the larger share.
# (GPSIMD cannot touch PSUM — neither directly nor via DMA staging — so it
# cannot help with any attention-phase work.)
ACT_COLS = 720
# Schraudolph: bf16_bits(exp(S/8)) ~= int16(S * (2^7/ln2)/8 + 127*2^7 - C)
EXP_A = 184.66496423378 / 8.0
EXP_B = 16250.5

NBF = 2 * C + 1 + C   # gt2 | wb | nxw columns in the bf16 const blob


def _patch_act_tables():
    """Bias the ACT-table-load insertion pass so Ln and Exp both resolve to
    the one act_info table that contains them both
    (natural_log_exp_and_others).  Table order/indices are untouched (walrus
    maps act_func_set_id by index); we only narrow the pass's coverage view,
    so the single load happens at the early dummy Ln and nothing reloads on
    the critical path."""
    from concourse import hw_specs
    orig = hw_specs.get_activation_tables

    def patched(arch):
        t = {name: set(fns) for name, fns in orig(arch).items()}
        for name, fns in t.items():
            if name != "natural_log_exp_and_others":
                fns.discard(AF.Exp)
                fns.discard(AF.Ln)
        return t

    bacc.get_activation_tables = patched


def build_graph():
    _patch_act_tables()
    nc = bacc.Bacc("TRN2", target_bir_lowering=False, debug=False,
                   num_devices=N_CORES)

    xt2_ext = nc.declare_dram_parameter("xt2", [128, N // 2], BF16,
                                        isOutput=False)
    wbf_ext = nc.declare_dram_parameter("wbf", [C, NBF], BF16, isOutput=False)
    wf32_ext = nc.declare_dram_parameter("wf32", [C, 2], F32, isOutput=False)
    g128_ext = nc.declare_dram_parameter("g128", [128, 1], F32, isOutput=False)
    wsr2_ext = nc.declare_dram_parameter("wsr2", [128, NTAP * C], BF16,
                                         isOutput=False)
    out_ext = nc.declare_dram_parameter("out", [N, C], BF16, isOutput=True)

    with tile.TileContext(nc) as tc:
        with tc.tile_pool(name="const", bufs=1) as const_pool, \
             tc.tile_pool(name="persist", bufs=1) as persist_pool, \
             tc.tile_pool(name="work", bufs=2) as work_pool:

            # ---------- constants first: gpsimd/scalar queues start hot ----
            identity = const_pool.tile([128, 128], F32)
            make_identity(nc, identity[:])

            eps_t = const_pool.tile([1, 1], F32, tag="eps")
            nc.gpsimd.memset(eps_t[:], EPS)
            # dummy Ln: pre-loads the natural_log table early; the LN-phase
            # rstd=exp(-0.5*ln(var)) then pays exactly one table swap (to
            # exp), after which attention needs no more loads.
            warm_t = const_pool.tile([1, 1], F32, tag="warm")
            nc.scalar.activation(warm_t[:], eps_t[:], AF.Ln)

            junk_mv = const_pool.tile([128, 256], BF16, tag="junkmv")
            nc.gpsimd.memset(junk_mv[:], 1.0)
            junk_st = const_pool.tile([128, 128], BF16, tag="junkst")
            nc.gpsimd.memset(junk_st[:], 0.001)

            # stats stationary: 65 rows of 1/C (row 64 weights the eps row)
            ones65_st = const_pool.tile([C + 1, 1], F32, tag="ones65_st")
            nc.gpsimd.memset(ones65_st[:], 1.0 / C)
            ones65 = const_pool.tile([C + 1, 1], BF16, tag="ones65")
            nc.vector.tensor_copy(ones65[:], ones65_st[:])
            onesr_st = const_pool.tile([1, 128], F32, tag="onesr_st")
            nc.gpsimd.memset(onesr_st[:], 1.0)
            onesr1 = const_pool.tile([1, 128], BF16, tag="onesr1")
            nc.vector.tensor_copy(onesr1[:], onesr_st[:])

            # ---------- DMAs (all HWDGE via sync; 1 descriptor/partition) --
            # All transfers go out together: a single in-flight transfer
            # only sustains ~170GB/s, so serializing (gating) LOSES
            # aggregate bandwidth.  Concurrent, everything lands ~15us.
            HD = N // 4  # 4096 cols per x half (conv blocks 0-3 / 4-7)
            xlo = persist_pool.tile([128, HD], BF16, tag="xlo")
            xhi = persist_pool.tile([128, HD], BF16, tag="xhi")
            wsr_sb = const_pool.tile([128, NTAP, C], BF16, tag="wsr")
            wbf_sb = const_pool.tile([C, NBF], BF16, tag="wbf")
            wf32_sb = const_pool.tile([C, 2], F32, tag="wf32")
            g128_sb = const_pool.tile([128, 1], F32, tag="g128")
            nc.sync.dma_start(wsr_sb[:].rearrange("p t c -> p (t c)"),
                              wsr2_ext[:])
            nc.sync.dma_start(xlo[:], xt2_ext[:, 0:HD])
            nc.sync.dma_start(wbf_sb[:], wbf_ext[:])
            nc.sync.dma_start(wf32_sb[:], wf32_ext[:])
            nc.sync.dma_start(xhi[:], xt2_ext[:, HD:2 * HD])
            nc.sync.dma_start(g128_sb[:], g128_ext[:])

            gt2_sb = wbf_sb[:, 0:2 * C]
            wb_sb = wbf_sb[:, 2 * C:2 * C + 1]
            nxw_sb = wbf_sb[:, 2 * C + 1:NBF]
            bvp_sb = wf32_sb[:, 0:1]
            bsr_sb = wf32_sb[:, 1:2]

            xhv = [x[:].rearrange(
                "p (b jp dh i1 di jh) -> p b jp dh i1 di jh",
                b=4, jp=2, dh=4, i1=2, di=8, jh=8) for x in (xlo, xhi)]

            ov = out_ext[:].rearrange("(b p ur j) f -> b p ur j f",
                                      b=8, p=TOK_TILE, ur=8, j=2)

            vps = [persist_pool.tile([TOK_TILE, C + 1], BF16, tag=f"vps{h}",
                                     name=f"vps{h}")
                   for h in range(2)]

            with tc.tile_pool(name="pre_psum", bufs=2, space="PSUM") as pre_ps:
                # Every per-half intermediate gets its OWN tile: the Tile
                # dependency tracker is tile-granular, so sharing one tile
                # across halves falsely serializes half 0's chain behind
                # half 1's writers (measured: +2.5us on the LN tail).
                xs_ps = [pre_ps.tile([C, 128], F32, tag=f"conv{h}", bufs=1,
                                     name=f"conv{h}") for h in range(2)]
                kqr_ps = [pre_ps.tile([128, 128], F32, tag=f"kqr{h}", bufs=1,
                                      name=f"kqr{h}") for h in range(2)]
                m12_ps = [pre_ps.tile([1, 2, 128], F32, tag=f"m12{h}", bufs=1,
                                      name=f"m12{h}") for h in range(2)]
                ab_ps = [pre_ps.tile([128, 2, 128], F32, tag=f"abp{h}",
                                     bufs=1, name=f"abp{h}") for h in range(2)]

                # PE warm-up: a bf16 junk-matmul bridge (into abp[0], which
                # is overwritten later anyway) keeps the HAM clock gate busy
                # from engine start until conv's DMA data lands, so conv
                # runs at 2.4 GHz.  (f32 junk lowers to 2 HW passes and ran
                # cold-paced; bf16 N=256 is one 213ns-cold/107ns-warm pass.)
                junk_out = ab_ps[0][:].rearrange("p s b -> p (s b)")
                for _ in range(26):
                    nc.tensor.matmul(junk_out, junk_st[:], junk_mv[:],
                                     start=True, stop=True)

                def conv_half(bh):
                    # taps over x half bh -> kv columns [128bh, +128)
                    xv = xhv[bh]
                    for k, dj in enumerate(range(0, SR, 2)):
                        for di in range(SR):
                            tap = di * (SR // 2) + dj // 2
                            nc.tensor.matmul(
                                xs_ps[bh][:],
                                wsr_sb[:, tap, :],
                                xv[:, :, :, dj // 2, :, di, :],
                                start=(k == 0 and di == 0),
                                stop=(k == SR // 2 - 1 and di == SR - 1))

                # ---------- layernorm, fully per-half pipelined ----------
                # xs2_h = [xs | xs^2] bf16; row 64 = [0 | eps*C] so the 1/C
                # stationary yields m12_h = [mu | E[x^2]+eps] in one matmul.
                # Half 0's whole chain runs while conv half 1 is on the PE.
                xs2 = [work_pool.tile([C + 1, 2, 128], BF16, tag=f"sq{h}",
                                      name=f"sq{h}") for h in range(2)]
                kqraw = [work_pool.tile([128, 128], BF16, tag=f"kqraw{h}",
                                        name=f"kqraw{h}") for h in range(2)]
                ab = [work_pool.tile([1, 2, 128], BF16, tag=f"ab{h}",
                                     name=f"ab{h}") for h in range(2)]
                kq2h = [persist_pool.tile([128, 128], BF16, tag=f"kq2{h}",
                                          name=f"kq2{h}") for h in range(2)]
                xsn = work_pool.tile([C, NKV], BF16, tag="xsn")
                for h in range(2):
                    nc.gpsimd.memset(xs2[h][C:C + 1, 0, :], 0.0)
                    nc.gpsimd.memset(xs2[h][C:C + 1, 1, :], EPS * C)

                def ln_front(h):
                    # emitted right after conv_half(h): xs/xs^2 + the two
                    # PE ops (kqraw projection, stats matmul)
                    nc.vector.tensor_scalar_add(xs2[h][0:C, 0, :],
                                                xs_ps[h][:], bsr_sb)
                    nc.scalar.activation(xs2[h][0:C, 1, :], xs_ps[h][:],
                                         AF.Square, bias=bsr_sb)
                    # kqraw = G @ xs (pre-LN); the LN affine lands later via
                    # kq2 = a*kqraw + gsum*b  (gsum = row-sums of G)
                    nc.tensor.matmul(kqr_ps[h][:], gt2_sb, xs2[h][0:C, 0, :],
                                     start=True, stop=True)
                    # h0's copy rides the idle ACT during conv half 1; h1's
                    # goes to DVE so it doesn't delay ACT's h1 stats chain.
                    if h == 0:
                        nc.scalar.copy(kqraw[h][:], kqr_ps[h][:])
                    else:
                        nc.vector.tensor_copy(kqraw[h][:], kqr_ps[h][:])
                    nc.tensor.matmul(
                        m12_ps[h][:].rearrange("p s b -> p (s b)"), ones65[:],
                        xs2[h][:].rearrange("p s b -> p (s b)"),
                        start=True, stop=True)

                def ln_mid(h):
                    # mu^2 -> var -> rstd=exp(-.5 ln var) -> b=-mu*rstd
                    mu2 = work_pool.tile([1, 128], F32, tag=f"mu2{h}",
                                         name=f"mu2{h}")
                    nc.scalar.activation(mu2[:], m12_ps[h][:, 0, :],
                                         AF.Square)
                    var = work_pool.tile([1, 128], F32, tag=f"var{h}",
                                         name=f"var{h}")
                    nc.vector.scalar_tensor_tensor(
                        var[:], m12_ps[h][:, 1, :], 1.0, mu2[:],
                        op0=ALU.mult, op1=ALU.subtract)
                    lnv = work_pool.tile([1, 128], F32, tag=f"lnv{h}",
                                         name=f"lnv{h}")
                    nc.scalar.activation(lnv[:], var[:], AF.Ln)
                    nc.scalar.activation(ab[h][:, 0, :], lnv[:], AF.Exp,
                                         scale=-0.5)
                    nc.vector.scalar_tensor_tensor(
                        ab[h][:, 1, :], m12_ps[h][:, 0, :], -1.0,
                        ab[h][:, 0, :], op0=ALU.mult, op1=ALU.mult)

                def ln_tail(h):
                    # broadcast a|b to 128 partitions; stage to SBUF at once
                    # (ACT, idle here) so the PSUM bank frees immediately --
                    # the attention pools reuse these banks, and a late
                    # reader of abp blocks the first S matmul on a bank WAR.
                    if h == 1:
                        # dep-timed junk: executes when ab[1] lands (~right
                        # before ab2-1), bridging the LN-tail PE idle so the
                        # first attention chunks run at 2.4 GHz
                        nc.tensor.matmul(kqr_ps[1][:], onesr1[0:1, :],
                                         ab[1][0:1, 0, :],
                                         start=True, stop=True)
                    nc.tensor.matmul(
                        ab_ps[h][:].rearrange("p s b -> p (s b)"), onesr1[:],
                        ab[h][:].rearrange("p s b -> p (s b)"),
                        start=True, stop=True)
                    t1 = work_pool.tile([128, 128], F32, tag=f"t1kq{h}",
                                        name=f"t1kq{h}")
                    nc.vector.tensor_mul(t1[:], kqraw[h][:], ab_ps[h][:, 0, :])
                    ab_sb = work_pool.tile([128, 2, 128], BF16,
                                           tag=f"absb{h}", name=f"absb{h}")
                    nc.scalar.copy(ab_sb[:], ab_ps[h][:])
                    nc.vector.scalar_tensor_tensor(
                        kq2h[h][:], ab_ps[h][:, 1, :], g128_sb[:], t1[:],
                        op0=ALU.mult, op1=ALU.add)
                    cs = slice(128 * h, 128 * h + 128)
                    nc.vector.tensor_mul(xsn[:, cs], xs2[h][0:C, 0, :],
                                         ab_sb[0:C, 0, :])
                    nc.vector.tensor_add(xsn[:, cs], xsn[:, cs],
                                         ab_sb[0:C, 1, :])
                    if h == 0:
                        # free-running junks keep the PE's HAM activity up
                        # through the serial h1 stats chain
                        for _ in range(6):
                            nc.tensor.matmul(kqr_ps[0][:], junk_st[:],
                                             junk_mv[:, 0:128],
                                             start=True, stop=True)

                conv_half(0)
                ln_front(0)
                conv_half(1)
                ln_mid(0)
                ln_front(1)
                ln_tail(0)
                ln_mid(1)
                ln_tail(1)

            # ---------- attention ----------
            # chunk ci = xT2 free block [256ci, 256ci+256): even-parity tokens
            # on partitions 0:64, odd on 64:128.  E col layout per chunk:
            # par*512 + mh*256 + tok.  y(ci-1) is emitted after S(ci)/exp(ci)
            # so the in-order PE queue never waits on exp before S(ci+1).
            def emit_s_exp(ci, s_pool, e_pool):
                s_ps = s_pool.tile([TOK_TILE, 2 * N_CHUNK], F32, tag="S")
                if ci % 4 == 0:
                    # periodic junk matmul (overwritten by the S par=0
                    # matmuls via start=True): a chunky activity burst for
                    # the HAM clock gate, which otherwise re-throttles the
                    # PE on the borderline K=64/N=65 mix (removal: +2.8us).
                    nc.tensor.matmul(s_ps[:, 0:256], kq2h[0][:],
                                     xlo[:, 0:256], start=True, stop=True)
                xh = (xlo, xhi)[ci // 16]
                xb = xh[:, 256 * (ci % 16):256 * (ci % 16 + 1)]
                # mh-outer: consecutive par0/par1 matmuls sit on disjoint PE
                # row groups (tile_position rows 0/64 via base_partition) and
                # write different PSUM banks, so each pair runs concurrently.
                for mh in range(2):
                    for par in range(2):   # bank `par`: tokens of parity par
                        o = C * par
                        base = par * N_CHUNK + mh * 256
                        nc.tensor.matmul(s_ps[:, base:base + 256],
                                         kq2h[mh][o:o + C, :],
                                         xb[o:o + C, :], start=True, stop=True)
                e_t = e_pool.tile([TOK_TILE, 2 * N_CHUNK], BF16, tag="E",
                                  bufs=5)
                nc.scalar.activation(e_t[:, 0:ACT_COLS], s_ps[:, 0:ACT_COLS],
                                     AF.Exp, scale=0.125)
                nc.vector.tensor_scalar(
                    e_t[:, ACT_COLS:2 * N_CHUNK].bitcast(I16),
                    s_ps[:, ACT_COLS:2 * N_CHUNK],
                    EXP_A, EXP_B, op0=ALU.mult, op1=ALU.add)
                return e_t

            def emit_vps_chain(scratch):
                """V-path: PSUM carved from `scratch` (one free y bank)."""
                d_h = []
                for h in range(2):
                    bqk_ps = scratch[:, 384 + h:385 + h]
                    nc.tensor.matmul(bqk_ps,
                                     xsn[:, h * 128:(h + 1) * 128],
                                     wb_sb, start=True, stop=True)
                    dh = work_pool.tile([TOK_TILE, 1], F32, tag="dh")
                    nc.scalar.activation(dh[:], bqk_ps, AF.Exp, scale=0.125)
                    d_h.append(dh)
                # all vector work on ACT (Identity with bias/scale APs): the
                # DVE queue stays clear for the chunk-0..2 Schraudolphs, so
                # the s_ps ring releases on time during the pipeline ramp.
                vpT_ps = scratch[0:C, 128:384]
                nc.tensor.matmul(vpT_ps, nxw_sb, xsn[:], start=True, stop=True)
                vpT = work_pool.tile([C, NKV], F32, tag="vT_b")
                nc.vector.tensor_scalar_add(vpT[:], vpT_ps, bvp_sb)
                for h in range(2):
                    vpt_ps = scratch[:, 64 * h:64 * (h + 1)]
                    nc.tensor.transpose(vpt_ps, vpT[:, h * 128:(h + 1) * 128],
                                        identity[0:C, 0:C])
                    nc.vector.tensor_scalar(vps[h][:, 0:C], vpt_ps,
                                            d_h[h][:], None, op0=ALU.mult)
                    nc.vector.tensor_copy(vps[h][:, C:C + 1], d_h[h][:])

            def emit_y(ci, y_ps):
                e_t = e_tiles[ci]
                for u in range(4):
                    ysl = y_ps[:, ci % 2, u * (C + 1):(u + 1) * (C + 1)]
                    b, j = u // 2, u % 2
                    col0 = 512 * j + 128 * b
                    nc.tensor.matmul(ysl, e_t[:, col0:col0 + 128],
                                     vps[0][:], start=True, stop=False)
                    nc.tensor.matmul(ysl, e_t[:, 256 + col0:256 + col0 + 128],
                                     vps[1][:], start=False, stop=True)

            def emit_norm(ci, y_ps):
                # normalize chunks (ci-1, ci) and DMA out
                yv = y_ps[:, :, 0:4 * (C + 1)].rearrange(
                    "p t (u q) -> p t u q", u=4)
                r_t = work_pool.tile([TOK_TILE, 2, 4, 1], F32, tag="r", bufs=2)
                nc.vector.reciprocal(r_t[:], yv[:, :, :, C:C + 1])
                y_t = work_pool.tile([TOK_TILE, 8, C], BF16, tag="y", bufs=2)
                nc.vector.tensor_mul(
                    y_t[:].rearrange("p (t u) f -> p t u f", t=2),
                    yv[:, :, :, 0:C],
                    r_t[:].broadcast_to([TOK_TILE, 2, 4, C]))
                g = ci // 2
                nc.sync.dma_start(
                    ov[g // 2, :, 4 * (g % 2):4 * (g % 2) + 4, :, :],
                    y_t[:].rearrange("p (s j) f -> p s j f", s=4, j=2))

            # y lags S/exp by TWO chunks: y(ci-2)'s E finished a whole chunk
            # ago, so the in-order PE queue never stalls on the current exp.
            # PSUM: 2 S tiles (4 banks) + ~2 y tiles (4 banks) = 8.
            with tc.tile_pool(name="attn_psum_s", bufs=2, space="PSUM") as att_s, \
                 tc.tile_pool(name="attn_psum_y", bufs=1, space="PSUM") as att_y:
                e_tiles = {}
                y_tiles = {}

                def y_tile(k):
                    if k not in y_tiles:
                        y_tiles[k] = att_y.tile([TOK_TILE, 2, 512], F32,
                                                tag="Y", name=f"y{k}")
                    return y_tiles[k]

                def emit_y_norm(k):
                    emit_y(k, y_tile(k // 2))
                    if k % 2 == 1:
                        emit_norm(k, y_tile(k // 2))

                def emit_norm_single(k, y_ps):
                    # tail-only: normalize+DMA one chunk so the final DMA
                    # isn't serialized behind a 2-chunk norm
                    t = k % 2
                    yv = y_ps[:, t:t + 1, 0:4 * (C + 1)].rearrange(
                        "p t (u q) -> p t u q", u=4)
                    r_t = work_pool.tile([TOK_TILE, 1, 4, 1], F32, tag="r1",
                                         bufs=2)
                    nc.vector.reciprocal(r_t[:], yv[:, :, :, C:C + 1])
                    y_t = work_pool.tile([TOK_TILE, 4, C], BF16, tag="y1",
                                         bufs=2)
                    nc.vector.tensor_mul(
                        y_t[:].rearrange("p (t u) f -> p t u f", t=1),
                        yv[:, :, :, 0:C],
                        r_t[:].broadcast_to([TOK_TILE, 1, 4, C]))
                    g = k // 2
                    nc.sync.dma_start(
                        ov[g // 2, :,
                           4 * (g % 2) + 2 * t:4 * (g % 2) + 2 * t + 2, :, :],
                        y_t[:].rearrange("p (s j) f -> p s j f", s=2, j=2))

                for ci in range(N_CHUNKS):
                    # y/norm of ci-2 go FIRST: their E is long done, the PE
                    # queue can't stall, and the norm mul runs on DVE while
                    # the PE computes S(ci) -- emitted after S it queue-
                    # blocks schrau(ci+1) and the next s-bank release.
                    if ci >= 2:
                        emit_y_norm(ci - 2)
                    e_tiles[ci] = emit_s_exp(ci, att_s, work_pool)
                    if ci == 0:
                        # vps chain uses the (empty) second bank of y tile 0
                        emit_vps_chain(y_tile(0)[:, 1, :])
                    e_tiles.pop(ci - 3, None)
                k = N_CHUNKS - 2
                emit_y(k, y_tile(k // 2))
                emit_norm_single(k, y_tile(k // 2))
                emit_y(k + 1, y_tile(k // 2))
                emit_norm_single(k + 1, y_tile(k // 2))

    nc.finalize()
    return nc


_NC_CACHE = None


def _get_nc():
    global _NC_CACHE
    if _NC_CACHE is None:
        _NC_CACHE = build_graph()
    return _NC_CACHE


def _fold_weights(inputs):
    """Host-side weight folding (all pure functions of the weights)."""
    f32 = np.float32
    Wq = np.asarray(inputs["Wq"], f32)
    Wk = np.asarray(inputs["Wk"], f32)
    Wv = np.asarray(inputs["Wv"], f32)
    Wp = np.asarray(inputs["Wp"], f32)
    Wsr = np.asarray(inputs["Wsr"], f32)
    bq = np.asarray(inputs["bq"], f32)
    bv = np.asarray(inputs["bv"], f32)
    bsr = np.asarray(inputs["bsr"], f32)
    bp = np.asarray(inputs["bp"], f32)
    gamma = np.asarray(inputs["gamma"], f32)
    beta = np.asarray(inputs["beta"], f32)

    bf = ml_dtypes.bfloat16
    Wkg = gamma[:, None] * Wk
    Wvg = gamma[:, None] * Wv
    G = Wq @ Wkg.T                                   # [C, C]
    wbf = np.concatenate(
        [G.T, G.T, (Wkg @ bq)[:, None], Wvg @ Wp], axis=1)
    wbf = np.ascontiguousarray(wbf, dtype=bf)        # [C, 2C+1+C]
    bvp = (beta @ Wv + bv) @ Wp + bp
    wf32 = np.ascontiguousarray(
        np.stack([bvp, bsr], axis=1), dtype=f32)     # [C, 2]
    # gsum[p] = sum_c gt2[c, p] = row-sums of G (for the kq2 LN-affine fold)
    g128 = np.ascontiguousarray(
        np.tile(G.sum(axis=1), 2)[:, None], dtype=f32)  # [128, 1]

    # wsr2: packed tap-pairs — slot k holds even tap 2k on partitions 0:64
    # and its odd partner 2k+1 on partitions 64:128, so K=128 matmuls fuse
    # tap pairs across the token-parity partition split (only even taps are
    # ever loaded as stationary; the old layout shipped 2x the bytes).
    wsr_f = Wsr.reshape(SR * SR, C, C)               # [tap, cin, cout]
    wsr2 = np.zeros((128, NTAP, C), dtype=bf)
    wsr2[0:C] = np.swapaxes(wsr_f[0::2], 0, 1)       # [cin, pair, cout]
    wsr2[C:128] = np.swapaxes(wsr_f[1::2], 0, 1)
    wsr2 = np.ascontiguousarray(wsr2.reshape(128, NTAP * C))
    return dict(wbf=wbf, wf32=wf32, wsr2=wsr2, g128=g128)


def _make_in_maps(inputs):
    x = np.asarray(inputs["x"], dtype=np.float32)
    B = x.shape[0]
    assert x.shape == (B, N, C) and B == N_CORES, x.shape
    common = _fold_weights(inputs)
    x_bf = np.asarray(x, dtype=ml_dtypes.bfloat16)
    in_maps = []
    for i in range(N_CORES):
        # xT2[jp*64+c, (blk*8+pj)*128+p] = x[blk*2048 + p*16 + pj*2 + jp, c]
        x3 = x_bf[i].reshape(8, 128, 8, 2, C)        # [blk, p, pj, jp, c]
        xt2 = np.ascontiguousarray(
            x3.transpose(3, 4, 0, 2, 1).reshape(128, N // 2))
        in_maps.append(dict(common, xt2=xt2))
    return in_maps


def run(inputs, trace=False):
    nc = _get_nc()
    in_maps = _make_in_maps(inputs)
    res = run_bass_kernel_spmd(nc, in_maps, list(range(N_CORES)), trace=trace)
    out = np.stack([np.asarray(res.results[i]["out"]) for i in range(N_CORES)])
    return out.astype(np.float32), res


def kernel(**inputs):
    out, _ = run(inputs, trace=False)
    return out

